# revision 6
# baseline (speedup 1.0000x reference)
"""NT-Xent loss kernel for Trainium2, 8 NeuronCores.

Problem: B=4096 per view, D=128, temperature=0.1.
reps = concat([zjs, zis]) -> [8192, 128]; normalize rows; sim = normed @ normed.T;
loss = mean_i(-pos_i/T + logsumexp_{j!=i}(sim_ij/T)).

Strategy (fully static SPMD, no collectives) — exploits sim symmetry to
halve the exp work vs a full row-block scan:
  The 8192 rows form 64 tiles of 128.  Row tile T computes only the
  column band [T, T+32] (33 tiles, contiguous in the per-core rotated
  frame): the diagonal tile contributes row sums only; tiles T+1..T+31
  contribute row sums AND column sums (the transposed half of each
  pair); tile T+32 contributes both at host weight 0.5 (pairs at tile
  distance 32 are computed from both sides).  Every unordered pair then
  lands in r_i / r_j exactly once, so the exp covers the full matrix at
  half the cost.  Per core: its 8 row tiles (1024 rows); the rotated
  input keeps the band contiguous, so only 44 of 64 column tiles are
  touched.

  Host prep (O(N*D), ~0.01% of the O(N^2*D) device work): normalize
  rows in f32, quantize to bf16, ship the matmul operand in transposed
  [128d x rows] layout.  pos_i / diag_i dots are computed on the host
  from the same bf16 values (bf16->f32 is exact, so the host diag
  matches the device matmul's own self-column to f32 rounding).

  Device, per (row tile, strip in {1536,1536,1152} of the 4224 band):
    PE    sim matmuls -> PSUM  (stationary = row tile, moving = band)
    ACT   exp(10x-10) PSUM -> SBUF bf16 E   (pure exp stream — the
          bottleneck engine runs back-to-back activations)
    DVE   tensor_scalar accumulate row sums of E (+ separate tail sum)
    PE    per-512-chunk column sums: ONES [128,1] stationary, E chunk
          moving -> a [1,<=512] row vector written to its own partition
          of one shared [128,512] PSUM tile (72 chunks total).  This
          replaces per-tile E-stationary matmuls: no 128-cycle LDWEIGHTS
          per 128 columns, just one moving pass over E.
  Host combines row/col partials (0.5 weight on the distance-32 tail),
  subtracts exp(10 diag - 10), takes log and averages in f64.
"""

import numpy as np

B = 4096
D = 128
TWO_B = 2 * B
P = 128
NCORES = 8
ROWS_PER_CORE = TWO_B // NCORES  # 1024
MI = 8                    # row tiles per core (128 rows each)
NTILES_IN = 44            # band cols reach local tile 40; pad to 44
NSLICES = NTILES_IN // 4
STRIPS = ((0, 1536), (1536, 1536), (3072, 1152))
INV_T = 10.0              # 1 / temperature
SHIFT = 10.0              # fixed logsumexp shift (sim/T <= 10)
OUT_W = 32 + 512          # rowsum/tail block + colsum block

# colsum chunks per (tile, strip): (strip idx, start col, width)
# strip A chunk 0 starts at 128 to skip the diagonal tile's columns.
CHUNKS = []
for _si, (_off, _w) in enumerate(STRIPS):
    for _k in range(0, _w, 512):
        _s = 128 if (_si == 0 and _k == 0) else _k
        CHUNKS.append((_si, _s, min(512, _w - _k) - (_s - _k)))
NCH = len(CHUNKS)  # 9 per row tile

_CACHE = {}


def build_nc():
    import concourse.bacc as bacc
    import concourse.bass as bass
    import concourse.mybir as mybir
    import concourse.tile as tile

    f32 = mybir.dt.float32
    bf16 = mybir.dt.bfloat16
    OP = mybir.AluOpType
    AF = mybir.ActivationFunctionType

    # Pin the act-table chooser to the one set that holds Exp so no
    # mid-kernel ACT_TABLE_LOADs are emitted.
    from concourse import hw_specs

    _orig_tables = hw_specs.get_activation_tables

    def _patched_tables(arch):
        t = {k: set(v) for k, v in _orig_tables(arch).items()}
        for name, s in t.items():
            if name != "natural_log_exp_and_others":
                s.discard(AF.Exp)
                s.discard(AF.Ln)
        return t

    bacc.get_activation_tables = _patched_tables

    nc = bacc.Bacc(
        "TRN2",
        target_bir_lowering=False,
        debug=False,
        num_devices=NCORES,
    )
    # hit[d, 128t+p] = bf16(normed_rot[128t+p, d])  (transposed layout)
    hit_h = nc.declare_dram_parameter("hit", [P, NTILES_IN * P], bf16,
                                      isOutput=False)
    out_h = nc.declare_dram_parameter("out", [P, OUT_W], f32, isOutput=True)

    with tile.TileContext(nc) as tc:
        with (
            tc.tile_pool(name="persist", bufs=1) as persist,
            tc.tile_pool(name="psum", bufs=2, space="PSUM") as psum,
            tc.tile_pool(name="psumacc", bufs=1, space="PSUM") as psumacc,
            tc.tile_pool(name="escr", bufs=4) as escr,
        ):
            HIT = persist.tile([P, NTILES_IN * P], bf16)
            OUTBUF = persist.tile([P, 32], f32)
            # U[:, 0:71] = 0, U[:, 71] = 1: colsum chunk q uses stationary
            # U[:, 71-q:72] so its [1, w] result lands on out partition q
            # (matmul out base partition must be 0; rows 0..q-1 add zeros).
            U = persist.tile([P, 72], bf16)
            ZER = persist.tile([P, P], bf16)
            JB = persist.tile([P, 1536], bf16)
            JT = persist.tile([P, P], bf16)
            bias_shift = persist.tile([P, 1], f32)
            nc.vector.memset(U, 0.0)
            nc.vector.memset(U[:, 71:72], 1.0)
            nc.vector.memset(ZER, 0.0)
            nc.vector.memset(bias_shift, -SHIFT)

            # one PSUM bank: partition q holds colsum chunk q's row vector
            CS = psumacc.tile([P, 512], f32)
            CSOUT = persist.tile([P, 512], f32)

            # ---------------- loads + PE warm-up ----------------------------
            dmaq = [nc.gpsimd, nc.sync, nc.scalar]
            for s in range(NSLICES):
                x, y = 4 * s * P, (4 * s + 4) * P
                dmaq[s % 3].dma_start(out=HIT[:, x:y], in_=hit_h[:, x:y])
            # warm the PE p-state during the load phase (into CS); the final
            # warm-up uses a zero stationary so CS ends up zeroed for the
            # accumulating colsum chunks below.
            for _ in range(5):
                nc.tensor.matmul(
                    CS, HIT[:, 0:P], HIT[:, 0:512],
                    start=True, stop=True,
                )
            nc.tensor.matmul(
                CS, ZER, HIT[:, 0:512], start=True, stop=True,
            )

            # ---------------- strips: sim + exp + row/col sums --------------
            # Strip-major (all A, then B, then C) so early strips only need
            # early HIT slices.  Colsum matmuls queue one row tile behind the
            # sims; row sums all on DVE so ACT is a pure exp stream.
            pending_cs = []

            def flush_colsums():
                for lhs, q, wdt in pending_cs:
                    nc.tensor.matmul(
                        CS[0 : q + 1, 0:wdt], U[:, 71 - q : 72], lhs,
                        start=False, stop=True, skip_group_check=True,
                    )
                pending_cs.clear()

            for si, (off, w) in enumerate(STRIPS):
                for t in range(MI):
                    base = P * t
                    pg = psum.tile([P, 1536], f32, tag="pg")
                    for k in range(0, w, 512):
                        kw = min(512, w - k)
                        nc.tensor.matmul(
                            pg[:, k : k + kw],
                            HIT[:, base : base + P],
                            HIT[:, base + off + k : base + off + k + kw],
                            start=True, stop=True,
                        )
                    flush_colsums()
                    E = escr.tile([P, 1536], bf16, tag="e")
                    nc.scalar.activation(
                        out=E[:, :w], in_=pg[:, :w], func=AF.Exp,
                        scale=INV_T, bias=bias_shift,
                    )
                    nc.vector.tensor_scalar(
                        out=JB[:, :w], in0=E[:, :w], scalar1=1.0,
                        scalar2=0.0, op0=OP.mult, op1=OP.add,
                        accum_out=OUTBUF[:, 3 * t + si : 3 * t + si + 1],
                    )
                    if si == 2:
                        # tail tile (distance 32): separate row sum so the
                        # host can apply weight 0.5
                        nc.vector.tensor_scalar(
                            out=JT, in0=E[:, 1024:1152], scalar1=1.0,
                            scalar2=0.0, op0=OP.mult, op1=OP.add,
                            accum_out=OUTBUF[:, 24 + t : 25 + t],
                        )
                    for ci, (csi, s, wdt) in enumerate(CHUNKS):
                        if csi != si:
                            continue
                        pending_cs.append(
                            (E[:, s : s + wdt], t * NCH + ci, wdt)
                        )
            flush_colsums()

            # drain colsums: PSUM -> SBUF -> DRAM
            nc.vector.tensor_scalar(
                out=CSOUT, in0=CS, scalar1=1.0, scalar2=None, op0=OP.mult,
            )
            nc.sync.dma_start(out=out_h[:, 0:32], in_=OUTBUF)
            nc.sync.dma_start(out=out_h[:, 32:OUT_W], in_=CSOUT)

    nc.compile()
    return nc


def get_nc():
    if "nc" not in _CACHE:
        _CACHE["nc"] = build_nc()
    return _CACHE["nc"]


def _prep(zis: np.ndarray, zjs: np.ndarray):
    import ml_dtypes

    # representations in reference order: [zjs; zis], normalized rows
    # (f32 norms with the torch CosineSimilarity 1e-8 clamp)
    reps = np.concatenate(
        [np.asarray(zjs, np.float32), np.asarray(zis, np.float32)], axis=0
    )
    normed = (
        reps / np.maximum(np.linalg.norm(reps, axis=1, keepdims=True), 1e-8)
    ).astype(ml_dtypes.bfloat16)
    return normed


def make_in_maps(zis: np.ndarray, zjs: np.ndarray):
    normed = _prep(zis, zjs)
    maps = []
    for c in range(NCORES):
        rot = np.roll(normed, -ROWS_PER_CORE * c, axis=0)[: NTILES_IN * P]
        hit = np.ascontiguousarray(rot.T)            # [128 d, 5632 rows]
        maps.append({"hit": hit})
    return maps


def kernel(zis: np.ndarray, zjs: np.ndarray) -> np.ndarray:
    from concourse.bass_utils import run_bass_kernel_spmd

    nc = get_nc()
    normed = _prep(zis, zjs)
    maps = []
    for c in range(NCORES):
        rot = np.roll(normed, -ROWS_PER_CORE * c, axis=0)[: NTILES_IN * P]
        maps.append({"hit": np.ascontiguousarray(rot.T)})

    res = None
    for attempt in range(3):
        try:
            res = run_bass_kernel_spmd(nc, maps, core_ids=list(range(NCORES)))
            break
        except Exception:
            # transient device-unrecoverable states heal on re-execution
            if attempt == 2:
                raise
            import time as _time

            _time.sleep(5.0)

    # ---- host combine (f64) -------------------------------------------
    nf = normed.astype(np.float64)
    pos = np.sum(nf * np.roll(nf, -B, axis=0), axis=1)   # h_i . h_{(i+B)%2B}
    diag = np.sum(nf * nf, axis=1)

    r = np.zeros(TWO_B, dtype=np.float64)

    p_idx = np.arange(P)
    t_idx = np.arange(MI)
    row_l = 128 * t_idx[None, :] + p_idx[:, None]              # [P, MI]

    # colsum chunk q = t*NCH + ci covers rotated cols
    # 128*t + STRIPS[csi][0] + s .. + wdt, tail chunk (csi==2, s==1024)
    # carries host weight 0.5
    for c, rr in enumerate(res.results):
        o = rr["out"].astype(np.float64)                       # [P, OUT_W]
        rsum = o[:, 0:24].reshape(P, MI, 3)
        tail = o[:, 24:32]
        g_row = (1024 * c + row_l) % TWO_B
        np.add.at(r, g_row,
                  rsum[:, :, 0] + rsum[:, :, 1] + rsum[:, :, 2]
                  - 0.5 * tail)
        cs = o[:, 32:OUT_W]                                    # [128, 512]
        for t in range(MI):
            for ci, (csi, s, wdt) in enumerate(CHUNKS):
                q = t * NCH + ci
                gcol = (1024 * c + 128 * t + STRIPS[csi][0] + s
                        + np.arange(wdt)) % TWO_B
                wgt = 0.5 if (csi == 2 and s == 1024) else 1.0
                np.add.at(r, gcol, wgt * cs[q, :wdt])

    lse = np.log(r - np.exp(INV_T * diag - SHIFT)) + SHIFT
    loss = np.mean(-INV_T * pos + lse)
    return np.array(loss, dtype=np.float32)
